# revision 48
# baseline (speedup 1.0000x reference)
"""Trainium2 Bass kernel for nn_DotProductAttention_6030134084023.

reference: softmax(mask(Q @ K^T / sqrt(64), valid_lens)) @ V
  query/key/value: [64, 1024, 64] f32, valid_lens: [64] int32 -> [64, 1024, 64] f32

Strategy
--------
Batch dim sharded across 8 NeuronCores; host sorts batches by valid_len
(descending) and deals them round-robin so slot s holds similar lengths on
every core; kernel is compiled per distinct per-slot chunk-count tuple.

All matmuls in bf16 (host converts): f32r matmuls run in slow
fp32_mode=HIGH (~2 cy/row); bf16 streams 1 cy/row with fast weight load.
V is pre-masked on the host into Vm = [V*mask, mask x64] ([S, 128], SBUF
chunk layout) so there is no mask work on device AND the UT matmul writes
the softmax denominator REPLICATED on PSUM rows 64:128 -- the partition
broadcast the normalize needs happens inside the matmul for free.

Per batch (k-chunks of 128, exact count nreal = ceil(slot_max/128)):
  ST[k, q]   = KT_chunk.T @ QT    two chunks run concurrently as
                                  row-packed tiles (tile_position)
  EST        = exp(0.125 * ST)    split between ScalarE (exact exp ->
                                  bf16) and DVE (one fused Schraudolph
                                  op: i32(A*s + B); high 16 bits of the
                                  i32 ARE bf16 exp, read via stride-2 view)
  UT        += Vm_chunk.T @ EST   bf16 K=128, PSUM [128, 512] per q-half
postprocess per half (each stage 1 tick apart in the next batch's tile
stream, so the in-order DVE queue never blocks on cross-engine latency):
  den[64,512] <- UT rows 64:128 (ScalarE or DVE copy, greedy-balanced)
  rec = reciprocal_approx_fast(den)   (DVE; needs SBUF input -- PSUM
                                       input compiles but returns garbage)
  OT = UT[0:64] * rec -> bf16 SBUF -> DMA out; host converts to f32.

Emission uses a due-tick work queue: UT matmuls lag their exp tile by 3
ticks (est latency ~1.3us vs ~0.85us of PE work per tile) and postprocess
stages spread so ScalarE/DVE/PE all stay >60% busy.
"""

import numpy as np
import ml_dtypes

import concourse.bass as bass
import concourse.bacc as bacc
import concourse.tile as tile
from concourse import mybir
from concourse import bass_utils

F32 = mybir.dt.float32
BF16 = mybir.dt.bfloat16
I32 = mybir.dt.int32
AF = mybir.ActivationFunctionType
ALU = mybir.AluOpType

NCORES = 8
B = 64
S = 1024
D = 64
BPC = B // NCORES  # 8 batch slots per core
KC = S // 128  # max k-chunks
QH = 512

BF = ml_dtypes.bfloat16

# Schraudolph fast-exp constants: i32(A*s + B); high u16 of the i32 is
# bf16(exp(s/8)).  B centers the sawtooth error and folds in +2^15 so the
# 16-bit truncation rounds instead of floors.
LOG2E = 1.4426950408889634
SCH_A = float(np.float32(0.125 * LOG2E * (1 << 23)))
SCH_B = float(np.float32((127 << 23) - 0.043677 * (1 << 23) + (1 << 15)))

# engine-balance model (us per [128, 1024] tile)
_ACT_TILE_US = lambda wid: (wid + 352) / 1.2e3 + 0.10
# +0.5 accounts for the stride-2 est read slowing this tile's UT matmuls
_DVE_TILE_US = lambda wid: (wid + 120) * 1.0417e-3 + 0.15 + 0.25
_ACT_EVAC_US = (512 + 352) / 1.2e3 + 0.10  # den block copy on ScalarE, per half
_DVE_EVAC_US = (512 + 120) * 1.0417e-3 + 0.15  # den block copy on DVE, per half
_DVE_POST_US = 2.9  # per-batch recip x2 + normalize TT x2 on DVE

_BUILD_CACHE = {}

SLOT_ORDER = [0, 1, 2, 3, 4, 5, 6, 7]


def _plan(valid_lens):
    order = np.argsort(-valid_lens, kind="stable")
    nreals = []
    for s in range(BPC):
        slot_max = int(valid_lens[order[s * NCORES]])
        nreals.append(max(1, -(-slot_max // 128)))
    return order, tuple(nreals)


def _assign_engines(nreals):
    """Greedy ACT/DVE balance. Returns ({(b, h, p): is_dve}, {b: evac_is_dve})."""
    t_act = 1.4  # act table load
    t_dve = 0.0
    out = {}
    evac = {}
    for b in SLOT_ORDER:
        R = nreals[b]
        npairs = (R + 1) // 2
        for h in range(2):
            for p in range(npairs):
                wid = 1024 if 2 * p + 1 < R else 512
                ca, cd = _ACT_TILE_US(wid), _DVE_TILE_US(wid)
                if t_act + ca <= t_dve + cd:
                    t_act += ca
                    out[(b, h, p)] = False
                else:
                    t_dve += cd
                    out[(b, h, p)] = True
        if t_act + 2 * _ACT_EVAC_US <= t_dve + 2 * _DVE_EVAC_US:
            t_act += 2 * _ACT_EVAC_US
            evac[b] = False
        else:
            t_dve += 2 * _DVE_EVAC_US
            evac[b] = True
        t_dve += _DVE_POST_US
    return out, evac


def _build(nreals):
    nc = bacc.Bacc("TRN2", target_bir_lowering=False, debug=False, num_devices=NCORES)
    qt = nc.dram_tensor("qt", [BPC, D, S], BF16, kind="ExternalInput").ap()
    kt = nc.dram_tensor("kt", [BPC, D, S], BF16, kind="ExternalInput").ap()
    # vm pre-arranged on host into SBUF layout: [128, KC*128] per batch,
    # vm[b, p, c*128 + j] = Vm[b, c*128 + p, j] where Vm cols 0:64 = V*mask
    # and cols 64:128 = mask (replicated 64x) -> UT rows 64:128 all hold the
    # softmax denominator, i.e. the broadcast happens inside the matmul.
    vm = nc.dram_tensor("vm", [BPC, 128, KC * 128], BF16, kind="ExternalInput").ap()
    ot = nc.dram_tensor("ot", [BPC, D, S], BF16, kind="ExternalOutput").ap()

    is_dve, evac_dve = _assign_engines(nreals)

    with tile.TileContext(nc) as tc:
        with (
            tc.tile_pool(name="qk", bufs=2) as qkp,
            tc.tile_pool(name="vmp", bufs=3) as vmp,
            tc.tile_pool(name="estp", bufs=7) as estp,
            tc.tile_pool(name="post", bufs=3) as postp,
            tc.tile_pool(name="stp", bufs=2, space="PSUM") as stp,
            tc.tile_pool(name="utp", bufs=4, space="PSUM") as utp,
        ):
            # Deferred-work queue keyed by global tile tick.  UT matmuls lag
            # their exp by 2 ticks (est latency ~1.3us vs PE ~0.85us/tile);
            # the postprocess chain spreads over the next batch so the
            # cross-engine recip->bcast->mult latency never blocks DVE.
            work = []  # (due_tick, seq, batch, closure)
            state = {"tick": 0, "seq": 0}

            def sched(delay, batch, fn):
                work.append((state["tick"] + delay, state["seq"], batch, fn))
                state["seq"] += 1
                work.sort()

            def drain_due():
                while work and work[0][0] <= state["tick"]:
                    work.pop(0)[3]()

            def drain_batches_upto(bmax):
                rest = []
                for item in work:
                    if item[2] <= bmax:
                        item[3]()
                    else:
                        rest.append(item)
                work[:] = rest

            def drain_all():
                while work:
                    work.pop(0)[3]()

            for bi, b in enumerate(SLOT_ORDER):
                R = nreals[b]
                kw = R * 128
                npairs = (R + 1) // 2

                qt2 = qkp.tile([128, S], BF16, tag=f"qt{b % 2}")
                kt2 = qkp.tile([128, kw], BF16, tag=f"kt{b % 2}")
                vm_all = vmp.tile([128, R * 128], BF16, tag="vm")
                qtb = qt[b].bitcast(BF16)
                ktb = kt[b].bitcast(BF16)
                # qt on the sync queue, kt on the gpsimd queue: the two
                # dispatch streams run in parallel, halving input latency
                for half in (slice(0, 64), slice(64, 128)):
                    nc.sync.dma_start(out=qt2[half, :], in_=qtb)
                    nc.gpsimd.dma_start(out=kt2[half, 0:kw], in_=ktb[:, 0:kw])
                nc.gpsimd.dma_start(
                    out=vm_all[:],
                    in_=bass.AP(
                        tensor=vm.tensor,
                        offset=vm.offset + b * 128 * KC * 128,
                        ap=[[KC * 128, 128], [1, R * 128]],
                    ),
                )

                uts = {
                    h: utp.tile([128, QH], F32, tag="ut", name=f"ut{bi}_{h}")
                    for h in range(2)
                }

                def emit_ut(h, p, ebuf, dve, uts=uts, vm_all=vm_all, R=R):
                    ut = uts[h]
                    for kcl in range(2):
                        c = 2 * p + kcl
                        if c >= R:
                            continue
                        if dve:
                            ev = ebuf[:]
                            mov = bass.AP(
                                tensor=ev.tensor,
                                offset=ev.offset + kcl * 2 * QH + 1,
                                ap=[[ev.ap[0][0], 128], [2, QH]],
                            )
                        else:
                            mov = ebuf[:, kcl * QH : (kcl + 1) * QH]
                        nc.tensor.matmul(
                            ut[:],
                            vm_all[:, c * 128 : (c + 1) * 128],
                            mov,
                            start=(c == 0),
                            stop=(c == R - 1),
                        )

                # ---- postprocess closures: den arrives replicated on rows
                # 64:128, so the chain is evac -> recip -> mult, no broadcast ----
                den = postp.tile([D, 2 * QH], F32, tag="den")
                rec = postp.tile([D, 2 * QH], F32, tag="rec")
                osb = postp.tile([D, 2 * QH], BF16, tag="osb")

                def post_evac(h, uts=uts, den=den, dv=evac_dve[b]):
                    dh = den[:, h * QH : (h + 1) * QH]
                    if dv:
                        nc.vector.tensor_copy(out=dh, in_=uts[h][64:128, :])
                    else:
                        nc.scalar.copy(out=dh, in_=uts[h][64:128, :])

                def post_recip(h, den=den, rec=rec):
                    nc.vector.reciprocal_approx_fast(
                        rec[:, h * QH : (h + 1) * QH], den[:, h * QH : (h + 1) * QH]
                    )

                def post_tt(h, uts=uts, osb=osb, rec=rec):
                    nc.vector.tensor_tensor(
                        out=osb[:, h * QH : (h + 1) * QH],
                        in0=uts[h][0:D, :],
                        in1=rec[:, h * QH : (h + 1) * QH],
                        op=ALU.mult,
                    )

                def post_out(b=b, osb=osb):
                    nc.gpsimd.dma_start(out=ot[b], in_=osb[:])

                for h in range(2):
                    hs = slice(h * QH, (h + 1) * QH)
                    for p in range(npairs):
                        wid = 2 * QH if 2 * p + 1 < R else QH
                        st = stp.tile([128, 2 * QH], F32, tag="st")
                        nc.tensor.matmul(
                            st[:, 0:QH],
                            kt2[0:64, 2 * p * 128 : (2 * p + 1) * 128],
                            qt2[0:64, hs],
                            start=True,
                            stop=True,
                            tile_position=(0, 0),
                        )
                        if 2 * p + 1 < R:
                            nc.tensor.matmul(
                                st[:, QH : 2 * QH],
                                kt2[64:128, (2 * p + 1) * 128 : (2 * p + 2) * 128],
                                qt2[64:128, hs],
                                start=True,
                                stop=True,
                                tile_position=(64, 0),
                            )
                        if is_dve[(b, h, p)]:
                            ebuf = estp.tile([128, 4 * QH], BF16, tag="estB")
                            nc.vector.tensor_scalar(
                                out=ebuf[:].bitcast(I32)[:, 0:wid],
                                in0=st[:, 0:wid],
                                scalar1=SCH_A,
                                scalar2=SCH_B,
                                op0=ALU.mult,
                                op1=ALU.add,
                            )
                        else:
                            ebuf = estp.tile([128, 2 * QH], BF16, tag="estA")
                            nc.scalar.activation(
                                out=ebuf[:, 0:wid],
                                in_=st[:, 0:wid],
                                func=AF.Exp,
                                scale=0.125,
                            )
                        sched(
                            3, bi,
                            lambda h=h, p=p, ebuf=ebuf, dv=is_dve[(b, h, p)],
                            eu=emit_ut: eu(h, p, ebuf, dv),
                        )
                        state["tick"] += 1
                        drain_due()
                    # schedule this half's postprocess chain right after its
                    # last UT (due tick +3): evac, recip, mult one tick apart
                    sched(3, bi, lambda pe=post_evac, h=h: pe(h))
                    sched(4, bi, lambda pr=post_recip, h=h: pr(h))
                    sched(5, bi, lambda pt=post_tt, h=h: pt(h))
                    if h == 1:
                        sched(6, bi, post_out)
                # force batch bi-1's leftovers before batch bi+1 allocates tiles,
                # so pool rings (ut bufs=4 = 2 batches) never wrap over live reads
                drain_batches_upto(bi - 1)
            drain_all()

    nc.compile()
    return nc


def kernel(query, key, value, valid_lens):
    query = np.asarray(query, dtype=np.float32)
    key = np.asarray(key, dtype=np.float32)
    value = np.asarray(value, dtype=np.float32)
    valid_lens = np.asarray(valid_lens).astype(np.int32).reshape(B)
    assert query.shape == (B, S, D) and key.shape == (B, S, D)
    assert value.shape == (B, S, D)

    order, nreals = _plan(valid_lens)
    nc = _BUILD_CACHE.get(nreals)
    if nc is None:
        nc = _build(nreals)
        _BUILD_CACHE[nreals] = nc

    qt_f = query.transpose(0, 2, 1)  # [B, D, S] views
    kt_f = key.transpose(0, 2, 1)
    arange = np.arange(S)
    in_maps = []
    for c in range(NCORES):
        idx = [int(order[s * NCORES + c]) for s in range(BPC)]
        vmc = np.zeros((BPC, S, 128), dtype=np.float32)
        for s, bi in enumerate(idx):
            vl = int(valid_lens[bi])
            vmc[s, :vl, :D] = value[bi, :vl, :]
            vmc[s, :vl, D:] = 1.0
        # SBUF layout: [BPC, 128, KC*128]
        vmc = np.ascontiguousarray(
            vmc.reshape(BPC, KC, 128, 128).transpose(0, 2, 1, 3).reshape(
                BPC, 128, KC * 128
            )
        )
        in_maps.append(
            {
                "qt": np.ascontiguousarray(qt_f[idx]).astype(BF),
                "kt": np.ascontiguousarray(kt_f[idx]).astype(BF),
                "vm": vmc.astype(BF),
            }
        )

    res = bass_utils.run_bass_kernel_spmd(nc, in_maps, core_ids=list(range(NCORES)))

    out = np.empty((B, S, D), dtype=np.float32)
    for c in range(NCORES):
        otc = np.asarray(res.results[c]["ot"]).astype(np.float32)  # [BPC, D, S]
        for s in range(BPC):
            out[int(order[s * NCORES + c])] = otc[s].T
    return out
